# revision 12
# baseline (speedup 1.0000x reference)
"""AttentiveGRU2 Trainium2 Bass kernel.

Model (see reference):
  edge-softmax over incoming edges per dst node, attention-weighted
  gather of projected node features, segment-sum per dst, ELU, GRUCell.

Strategy (8 NeuronCores, SPMD, no collectives):
  * Host sorts edges by dst. Nodes are grouped into 392 windows of 128
    consecutive ids; each core owns 49 windows (6272 node slots).
  * Every window's edge list is padded to T_win tiles of 128 edges so the
    instruction stream is identical on all cores (SPMD); pad edges carry
    dst_local = -1 (zero one-hot row) and src = 0.
  * Softmax shift-invariance: a_e = exp(l_e)/sum exp(l_e) without the
    segment max (logits are N(0,1); exp is safe in fp32).
  * The per-edge division by the segment denominator is folded through the
    segment sum:  c_v = W @ (sum_e ex_e nf[src_e]) / (sum_e ex_e) + b.
    So the edge phase only needs, per 128-edge tile:
      - one indirect-DMA gather of nf rows (batched across windows)
      - one DVE op building the scaled one-hot O[e,j] = ex_e * (dstl_e==j)
      - two PE matmuls accumulating  psum_u += O.T @ G,  psum_d += O.T @ 1
  * Node phase per window: ctx~ = psum_u / max(psum_d,eps); c = ctx~ @ W_p.T
    (+ b_p via rank-1 ones matmul); context = elu(c); GRU cell; relu; DMA out.
"""

import numpy as np

V, E, F = 50000, 800000, 128
NC = 8
WPC = 49              # windows per core
NPC = WPC * 128       # 6272 node slots per core
WTOT = NC * WPC       # 392 windows total (covers 50176 >= V node slots)
WPB = 4               # windows per gather batch

_compiled = {}        # T_win -> compiled nc


def _build_nc(T_win):
    import concourse.bass as bass
    import concourse.bacc as bacc
    import concourse.mybir as mybir
    import concourse.tile as tile

    f32 = mybir.dt.float32
    i32 = mybir.dt.int32
    AF = mybir.ActivationFunctionType
    OP = mybir.AluOpType

    T = WPC * T_win   # edge tiles per core

    nc = bacc.Bacc("TRN2", target_bir_lowering=False, debug=False,
                   num_devices=NC)

    # ---- DRAM parameters ----
    srcw_d = nc.dram_tensor("srcw", [128, T], i32, kind="ExternalInput")
    dstl_d = nc.dram_tensor("dstl", [128, T], f32, kind="ExternalInput")
    logit_d = nc.dram_tensor("logit", [128, T], f32, kind="ExternalInput")
    table_d = nc.dram_tensor("table", [V, F], f32, kind="ExternalInput")
    nfT_d = nc.dram_tensor("nfT", [128, NPC], f32, kind="ExternalInput")
    nfR_d = nc.dram_tensor("nfR", [128, NPC], f32, kind="ExternalInput")
    wprojT_d = nc.dram_tensor("wprojT", [128, 128], f32, kind="ExternalInput")
    wihT_d = nc.dram_tensor("wihT", [128, 384], f32, kind="ExternalInput")
    whhT_d = nc.dram_tensor("whhT", [128, 384], f32, kind="ExternalInput")
    bproj_d = nc.dram_tensor("bproj", [1, 128], f32, kind="ExternalInput")
    brz_d = nc.dram_tensor("brz", [1, 256], f32, kind="ExternalInput")
    bni_d = nc.dram_tensor("bni", [1, 128], f32, kind="ExternalInput")
    bnh_d = nc.dram_tensor("bnh", [1, 128], f32, kind="ExternalInput")
    iota_d = nc.dram_tensor("iota", [128, 128], f32, kind="ExternalInput")
    ident_d = nc.dram_tensor("ident", [128, 128], f32, kind="ExternalInput")
    onesc_d = nc.dram_tensor("onesc", [128, 1], f32, kind="ExternalInput")
    onesr_d = nc.dram_tensor("onesr", [1, 128], f32, kind="ExternalInput")
    out_d = nc.dram_tensor("out", [NPC, 128], f32, kind="ExternalOutput")

    with tile.TileContext(nc) as tc:
        with (
            tc.tile_pool(name="const", bufs=1) as cpool,
            tc.tile_pool(name="gat", bufs=2) as gpool,
            tc.tile_pool(name="oh", bufs=3) as opool,
            tc.tile_pool(name="wrk", bufs=2) as wpool,
            tc.tile_pool(name="pedge", bufs=2, space="PSUM") as pe_pool,
            tc.tile_pool(name="pnode", bufs=1, space="PSUM") as pn_pool,
        ):
            def load(pool, name, dram, shape, dtype=f32):
                t = pool.tile(shape, dtype, tag=name)
                nc.sync.dma_start(t[:], dram[:])
                return t

            iota_sb = load(cpool, "iota", iota_d, [128, 128])
            ident_sb = load(cpool, "ident", ident_d, [128, 128])
            onesc_sb = load(cpool, "onesc", onesc_d, [128, 1])
            onesr_sb = load(cpool, "onesr", onesr_d, [1, 128])
            wproj_sb = load(cpool, "wproj", wprojT_d, [128, 128])
            wih_sb = load(cpool, "wih", wihT_d, [128, 384])
            whh_sb = load(cpool, "whh", whhT_d, [128, 384])
            bproj_sb = load(cpool, "bproj", bproj_d, [1, 128])
            brz_sb = load(cpool, "brz", brz_d, [1, 256])
            bni_sb = load(cpool, "bni", bni_d, [1, 128])
            bnh_sb = load(cpool, "bnh", bnh_d, [1, 128])
            srcw_sb = load(cpool, "srcw", srcw_d, [128, T], i32)
            dstl_sb = load(cpool, "dstl", dstl_d, [128, T])
            logit_sb = load(cpool, "logit", logit_d, [128, T])
            nfT_sb = load(cpool, "nfT", nfT_d, [128, NPC])
            nfR_sb = load(cpool, "nfR", nfR_d, [128, NPC])

            ex_sb = cpool.tile([128, T], f32, tag="ex")
            nc.scalar.activation(ex_sb[:], logit_sb[:], AF.Exp)

            n_batches = (WPC + WPB - 1) // WPB
            for b in range(n_batches):
                w0 = b * WPB
                nw = min(WPB, WPC - w0)
                nt = nw * T_win
                G = gpool.tile([128, WPB * T_win, 128], f32, tag="G")
                nc.gpsimd.indirect_dma_start(
                    out=G[:, 0:nt, :],
                    out_offset=None,
                    in_=table_d[:],
                    in_offset=bass.IndirectOffsetOnAxis(
                        ap=srcw_sb[:, w0 * T_win: w0 * T_win + nt],
                        axis=0,
                    ),
                )
                for wl in range(nw):
                    w = w0 + wl
                    psum_u = pe_pool.tile([128, 128], f32, tag="psum_u")
                    psum_d = pe_pool.tile([128, 1], f32, tag="psum_d")
                    for tl in range(T_win):
                        t = w * T_win + tl
                        M = opool.tile([128, 128], f32, tag="M")
                        nc.vector.tensor_scalar(
                            out=M[:], in0=iota_sb[:],
                            scalar1=dstl_sb[:, t:t + 1], scalar2=None,
                            op0=OP.subtract,
                        )
                        O = opool.tile([128, 128], f32, tag="O")
                        nc.vector.tensor_scalar(
                            out=O[:], in0=M[:],
                            scalar1=0.0, scalar2=ex_sb[:, t:t + 1],
                            op0=OP.is_equal, op1=OP.mult,
                        )
                        nc.tensor.matmul(
                            psum_u[:], lhsT=O[:],
                            rhs=G[:, wl * T_win + tl, :],
                            start=(tl == 0), stop=(tl == T_win - 1),
                        )
                        nc.tensor.matmul(
                            psum_d[:], lhsT=O[:], rhs=onesc_sb[:],
                            start=(tl == 0), stop=(tl == T_win - 1),
                        )

                    # ---- node phase for window w ----
                    den = wpool.tile([128, 1], f32, tag="den")
                    nc.vector.tensor_scalar(
                        out=den[:], in0=psum_d[:], scalar1=1e-30,
                        scalar2=None, op0=OP.max)
                    rec = wpool.tile([128, 1], f32, tag="rec")
                    nc.vector.reciprocal(rec[:], den[:])
                    ctx_t = wpool.tile([128, 128], f32, tag="ctx_t")
                    nc.vector.tensor_scalar(
                        out=ctx_t[:], in0=psum_u[:], scalar1=rec[:, 0:1],
                        scalar2=None, op0=OP.mult)

                    ptr = pn_pool.tile([128, 128], f32, tag="ptrc")
                    nc.tensor.transpose(ptr[:], ctx_t[:], ident_sb[:])
                    ctxT = wpool.tile([128, 128], f32, tag="ctxT")
                    nc.vector.tensor_copy(out=ctxT[:], in_=ptr[:])

                    psum_c = pn_pool.tile([128, 128], f32, tag="ptrc")
                    nc.tensor.matmul(psum_c[:], lhsT=ctxT[:], rhs=wproj_sb[:],
                                     start=True, stop=False)
                    nc.tensor.matmul(psum_c[:], lhsT=onesr_sb[:],
                                     rhs=bproj_sb[:], start=False, stop=True)

                    # elu(c) = max(c,0) + exp(min(c,0)) - 1
                    cmin = wpool.tile([128, 128], f32, tag="cmin")
                    nc.vector.tensor_scalar(out=cmin[:], in0=psum_c[:],
                                            scalar1=0.0, scalar2=None,
                                            op0=OP.min)
                    cexp = wpool.tile([128, 128], f32, tag="cexp")
                    nc.scalar.activation(cexp[:], cmin[:], AF.Exp)
                    crelu = wpool.tile([128, 128], f32, tag="crelu")
                    nc.vector.tensor_scalar(out=crelu[:], in0=psum_c[:],
                                            scalar1=0.0, scalar2=None,
                                            op0=OP.max)
                    cexp1 = wpool.tile([128, 128], f32, tag="cexp1")
                    nc.vector.tensor_scalar(out=cexp1[:], in0=cexp[:],
                                            scalar1=1.0, scalar2=None,
                                            op0=OP.subtract)
                    ctx = wpool.tile([128, 128], f32, tag="ctx")
                    nc.vector.tensor_tensor(out=ctx[:], in0=cexp1[:],
                                            in1=crelu[:], op=OP.add)

                    ptr2 = pn_pool.tile([128, 128], f32, tag="ptrc")
                    nc.tensor.transpose(ptr2[:], ctx[:], ident_sb[:])
                    ctxT2 = wpool.tile([128, 128], f32, tag="ctxT2")
                    nc.vector.tensor_copy(out=ctxT2[:], in_=ptr2[:])

                    nfT_tile = nfT_sb[:, w * 128:(w + 1) * 128]
                    # r/z pre-activations: gi+gh accumulated in one PSUM
                    psum_rz = pn_pool.tile([128, 256], f32, tag="psum_rz")
                    nc.tensor.matmul(psum_rz[:], lhsT=ctxT2[:],
                                     rhs=wih_sb[:, 0:256],
                                     start=True, stop=False)
                    nc.tensor.matmul(psum_rz[:], lhsT=nfT_tile,
                                     rhs=whh_sb[:, 0:256],
                                     start=False, stop=False)
                    nc.tensor.matmul(psum_rz[:], lhsT=onesr_sb[:],
                                     rhs=brz_sb[:], start=False, stop=True)
                    # i_n | h_n in two halves of one PSUM bank
                    psum_nh = pn_pool.tile([128, 256], f32, tag="psum_nh")
                    nc.tensor.matmul(psum_nh[:, 0:128], lhsT=ctxT2[:],
                                     rhs=wih_sb[:, 256:384],
                                     start=True, stop=False)
                    nc.tensor.matmul(psum_nh[:, 0:128], lhsT=onesr_sb[:],
                                     rhs=bni_sb[:], start=False, stop=True)
                    nc.tensor.matmul(psum_nh[:, 128:256], lhsT=nfT_tile,
                                     rhs=whh_sb[:, 256:384],
                                     start=True, stop=False)
                    nc.tensor.matmul(psum_nh[:, 128:256], lhsT=onesr_sb[:],
                                     rhs=bnh_sb[:], start=False, stop=True)

                    rzs = wpool.tile([128, 256], f32, tag="rzs")
                    nc.scalar.activation(rzs[:], psum_rz[:], AF.Sigmoid)
                    nt1 = wpool.tile([128, 128], f32, tag="nt1")
                    nc.vector.tensor_tensor(out=nt1[:], in0=rzs[:, 0:128],
                                            in1=psum_nh[:, 128:256],
                                            op=OP.mult)
                    nt2 = wpool.tile([128, 128], f32, tag="nt2")
                    nc.vector.tensor_tensor(out=nt2[:], in0=nt1[:],
                                            in1=psum_nh[:, 0:128],
                                            op=OP.add)
                    nn = wpool.tile([128, 128], f32, tag="nn")
                    nc.scalar.activation(nn[:], nt2[:], AF.Tanh)

                    nf_tile = nfR_sb[:, w * 128:(w + 1) * 128]
                    df = wpool.tile([128, 128], f32, tag="df")
                    nc.vector.tensor_tensor(out=df[:], in0=nf_tile, in1=nn[:],
                                            op=OP.subtract)
                    dz = wpool.tile([128, 128], f32, tag="dz")
                    nc.vector.tensor_tensor(out=dz[:], in0=df[:],
                                            in1=rzs[:, 128:256], op=OP.mult)
                    hh = wpool.tile([128, 128], f32, tag="hh")
                    nc.vector.tensor_tensor(out=hh[:], in0=dz[:], in1=nn[:],
                                            op=OP.add)
                    outt = wpool.tile([128, 128], f32, tag="outt")
                    nc.scalar.activation(outt[:], hh[:], AF.Relu)
                    nc.sync.dma_start(out_d[w * 128:(w + 1) * 128, :],
                                      outt[:])

    nc.compile()
    return nc


def _prep(edge_logits, node_feats, W_proj, b_proj, w_ih, w_hh, b_ih, b_hh,
          src, dst):
    """Host-side sharding: sort edges by dst, pad per window, build per-core
    input maps. Returns (T_win, in_maps)."""
    logits = np.asarray(edge_logits, np.float32).reshape(-1)
    src = np.asarray(src, np.int32)
    dst = np.asarray(dst, np.int32)

    order = np.argsort(dst, kind="stable")
    dst_s = dst[order]
    src_s = src[order]
    log_s = logits[order]

    win = dst_s // 128                        # window of each sorted edge
    counts = np.bincount(win, minlength=WTOT)
    T_win = int((counts.max() + 127) // 128)
    slots_per_win = T_win * 128

    starts = np.zeros(WTOT, np.int64)
    starts[1:] = np.cumsum(counts)[:-1]
    pos = np.arange(E, dtype=np.int64) - starts[win]
    flat = win.astype(np.int64) * slots_per_win + pos

    n_slots = WTOT * slots_per_win
    src_pad = np.zeros(n_slots, np.int32)
    dstl_pad = np.full(n_slots, -1.0, np.float32)
    log_pad = np.zeros(n_slots, np.float32)
    src_pad[flat] = src_s
    dstl_pad[flat] = (dst_s - win * 128).astype(np.float32)
    log_pad[flat] = log_s

    # [WTOT, T_win, 128] -> per core [128, WPC*T_win]
    def core_layout(a):
        a = a.reshape(WTOT, T_win, 128)
        out = []
        for k in range(NC):
            blk = a[k * WPC:(k + 1) * WPC]          # [WPC, T_win, 128]
            out.append(np.ascontiguousarray(
                blk.transpose(2, 0, 1).reshape(128, WPC * T_win)))
        return out

    src_cores = core_layout(src_pad)
    dstl_cores = core_layout(dstl_pad)
    log_cores = core_layout(log_pad)

    nf = np.asarray(node_feats, np.float32)
    nf_pad = np.zeros((NC * NPC, F), np.float32)
    nf_pad[:V] = nf

    table = np.ascontiguousarray(nf)
    wprojT = np.ascontiguousarray(np.asarray(W_proj, np.float32).T)
    wihT = np.ascontiguousarray(np.asarray(w_ih, np.float32).T)
    whhT = np.ascontiguousarray(np.asarray(w_hh, np.float32).T)
    bproj = np.asarray(b_proj, np.float32).reshape(1, 128)
    bih = np.asarray(b_ih, np.float32).reshape(384)
    bhh = np.asarray(b_hh, np.float32).reshape(384)
    brz = (bih[0:256] + bhh[0:256]).reshape(1, 256)
    bni = bih[256:384].reshape(1, 128)
    bnh = bhh[256:384].reshape(1, 128)
    iota = np.tile(np.arange(128, dtype=np.float32), (128, 1))
    ident = np.eye(128, dtype=np.float32)
    onesc = np.ones((128, 1), np.float32)
    onesr = np.ones((1, 128), np.float32)

    in_maps = []
    for k in range(NC):
        sl = nf_pad[k * NPC:(k + 1) * NPC]
        nfT = np.ascontiguousarray(sl.T)
        nfR = np.ascontiguousarray(
            sl.reshape(WPC, 128, 128).transpose(1, 0, 2).reshape(128, NPC))
        in_maps.append({
            "srcw": src_cores[k], "dstl": dstl_cores[k],
            "logit": log_cores[k], "table": table,
            "nfT": nfT, "nfR": nfR,
            "wprojT": wprojT, "wihT": wihT, "whhT": whhT,
            "bproj": bproj, "brz": brz, "bni": bni, "bnh": bnh,
            "iota": iota, "ident": ident,
            "onesc": onesc, "onesr": onesr,
        })
    return T_win, in_maps


def kernel(edge_logits, node_feats, W_proj, b_proj, w_ih, w_hh, b_ih, b_hh,
           src, dst):
    from concourse.bass_utils import run_bass_kernel_spmd

    T_win, in_maps = _prep(edge_logits, node_feats, W_proj, b_proj,
                           w_ih, w_hh, b_ih, b_hh, src, dst)
    if T_win not in _compiled:
        _compiled[T_win] = _build_nc(T_win)
    nc = _compiled[T_win]

    res = run_bass_kernel_spmd(nc, in_maps, list(range(NC)))
    full = np.concatenate([res.results[k]["out"] for k in range(NC)], axis=0)
    return np.ascontiguousarray(full[:V]).astype(np.float32)
